# revision 15
# baseline (speedup 1.0000x reference)
"""TRN2 Bass kernel: 16-head attention (B=4, S=2048, HID=1024), fp32 I/O.

Full inputs in, full output out. Head-parallel sharding across 8 cores:
core c handles batch c//2 and heads [8*(c%2), 8*(c%2)+8) — Q/K/V
projections only for those 8 heads (512 of 1024 hidden dims), attention
over the full sequence, and a partial output projection over its 512
hidden dims. The host sums the two partials per batch and adds bo
(row-parallel tensor parallelism; no device collectives).

Matmuls run in bf16 (PSUM accumulates fp32); rel-err stays ~4e-3 vs the
fp32 reference, dominated by input quantization. Mask semantics: the
reference masks QUERY rows with -1e9 before softmax, which yields a
uniform distribution for masked rows. Here masked q rows are zeroed on
the host (scores 0 -> uniform softmax), and the Q bias is applied
through the mask row so bq on masked rows stays zero.

Device pipeline per core (8 heads, dh=64):
  QT[f,r] = wqT.T @ qT_scaled + bq*s*m   (host pre-scales q by s*mask)
  KT[f,k] = wkT.T @ kT + bk
  V'[k,f] = vT.T @ wvT + bv, with a ones column per head
  scoresT[k,sq] = KT_h.T @ QT_h  -> exp on ScalarE -> PV psum += V'_h.T @ expS
  (PV row 64 = softmax denominator via the ones column.)
  H = PV[0:64] * (1/denom);  out_partial[r,:] = H.T @ woT_half
"""

from contextlib import ExitStack

import numpy as np

import concourse.bass as bass
import concourse.bacc as bacc
import concourse.mybir as mybir
import concourse.tile as tile
from concourse.bass_utils import run_bass_kernel_spmd

DT = mybir.dt
F32 = DT.float32
AF = mybir.ActivationFunctionType
ALU = mybir.AluOpType

# Problem constants (hardcoded per harness contract)
B, S, HID, NH, DH = 4, 2048, 1024, 16, 64
N_CORES = 8

# toggles (kept as globals so test.py can flip them)
USE_F32R = True
USE_BF16 = True  # wins over USE_F32R when set
SHARD = "head"
TRACE = False
LAST_RESULTS = [None]


class Cfg3:
    """Head-parallel sharding: core = (batch, head-group of 8)."""

    def __init__(self, HID=1024, NH=16, S=2048, use_bf16=True, use_f32r=True):
        self.HID, self.NH, self.S = HID, NH, S
        self.R = S              # full batch queries per core
        self.DH = 64
        self.HPG = NH // 2      # 8 heads per core
        self.FG = self.HPG * self.DH          # 512 hidden dims per core
        self.FCG = self.FG // 128             # 4 head-pair chunks
        self.IC = HID // 128                  # 8 contraction chunks
        self.NKC = S // 128                   # 16 key chunks
        self.SQB = 512
        self.NSQB = self.R // self.SQB        # 4
        self.WV = self.HPG * 65               # 520
        self.XW = 512
        if use_bf16:
            self.MMDT = DT.bfloat16
        else:
            self.MMDT = DT.float32r if use_f32r else DT.float32


def build3(nc: bass.Bass, cfg: Cfg3, reps: int = 1, rep_phase: str = "all",
           expmode: str = "bf16"):
    HID, R, S = cfg.HID, cfg.R, cfg.S
    IC, HPG, FG, FCG = cfg.IC, cfg.HPG, cfg.FG, cfg.FCG
    NKC, SQB, NSQB, WV, XW = cfg.NKC, cfg.SQB, cfg.NSQB, cfg.WV, cfg.XW
    MMDT = cfg.MMDT
    ESDT = DT.float32r if expmode == "f32r" else MMDT
    VPDT = DT.float32r if expmode == "f32r" else MMDT

    dp = nc.declare_dram_parameter
    qT = dp("qT", [HID, R], MMDT, isOutput=False)
    kT = dp("kT", [HID, S], MMDT, isOutput=False)
    vT = dp("vT", [HID, S], MMDT, isOutput=False)
    wqT = dp("wqT", [HID, FG], MMDT, isOutput=False)
    wkT = dp("wkT", [HID, FG], MMDT, isOutput=False)
    wvT = dp("wvT", [HID, FG], MMDT, isOutput=False)
    woT = dp("woT", [FG, HID], MMDT, isOutput=False)
    bqr = dp("bqr", [1, FG], MMDT, isOutput=False)
    bkr = dp("bkr", [1, FG], MMDT, isOutput=False)
    bvr = dp("bvr", [1, FG], MMDT, isOutput=False)
    maskf = dp("maskf", [1, R], F32, isOutput=False)
    out = dp("out", [R, HID], F32, isOutput=True)  # partial: host adds pair+bo

    with tile.TileContext(nc) as tc, ExitStack() as ctx:
        cpool = ctx.enter_context(tc.tile_pool(name="consts", bufs=1))
        bq_sb = cpool.tile([1, FG], MMDT, tag="bq")
        bk_sb = cpool.tile([1, FG], MMDT, tag="bk")
        bv_sb = cpool.tile([1, FG], MMDT, tag="bv")
        nc.sync.dma_start(bq_sb[:], bqr[:])
        nc.sync.dma_start(bk_sb[:], bkr[:])
        nc.sync.dma_start(bv_sb[:], bvr[:])
        ones_f32 = cpool.tile([1, XW], F32, tag="ones32")
        nc.vector.memset(ones_f32[:], 1.0)
        ones_row = cpool.tile([1, XW], MMDT, tag="ones")
        nc.vector.tensor_copy(ones_row[:], ones_f32[:])
        NOC = NKC * HPG  # ones-column count in V' (128)
        onesw_f32 = cpool.tile([128, NOC], F32, tag="onesw32")
        nc.vector.memset(onesw_f32[:], 1.0)
        ones_wide = cpool.tile([128, NOC], VPDT, tag="onesw")
        nc.vector.tensor_copy(ones_wide[:], onesw_f32[:])
        mrow_f32 = cpool.tile([1, R], F32, tag="mrow32")
        nc.sync.dma_start(mrow_f32[:], maskf[:])
        mrow = cpool.tile([1, R], MMDT, tag="mrow")
        nc.vector.tensor_copy(mrow[:], mrow_f32[:])
        dbounce = ctx.enter_context(
            tc.tile_pool(name="dbounce", bufs=4, space="DRAM"))

        gpool = ctx.enter_context(tc.tile_pool(name="gstore", bufs=1))
        h_tile = gpool.tile([128, FCG * R], MMDT, tag="h")

        # persistent pools so weight/activation DMAs prefetch across phases
        wpool = ctx.enter_context(tc.tile_pool(name="wgt", bufs=2))
        xpool = ctx.enter_context(tc.tile_pool(name="xin", bufs=2))
        wopool = ctx.enter_context(tc.tile_pool(name="wo", bufs=1))

        def pe_touch(ppool, ap):
            # 1x1 matmul that absorbs a DMA-queue wait into the PE clock, so
            # real matmuls stay within the 2-sync-wait ISA budget
            pt = ppool.tile([1, 1], F32, tag="pt", bufs=2)
            if MMDT == DT.float32r:
                ap = ap.bitcast(F32)
            nc.tensor.matmul(pt[:], ap, ap, start=True, stop=True)

        def load_w(wT, ppool):
            w_sb = wpool.tile([128, IC * FG], MMDT, tag="w")
            src = wT[:, :].rearrange("(i p) f -> p i f", p=128)
            nc.sync.dma_start(w_sb[:].rearrange("p (i f) -> p i f", i=IC), src)
            pe_touch(ppool, w_sb[0:1, 0:1])
            return w_sb

        def load_x(xT, rb, ppool):
            x_sb = xpool.tile([128, IC * XW], MMDT, tag="x")
            src = xT[:, rb * XW:(rb + 1) * XW].rearrange(
                "(i p) w -> p i w", p=128)
            nc.sync.dma_start(x_sb[:].rearrange("p (i w) -> p i w", i=IC), src)
            pe_touch(ppool, x_sb[0:1, 0:1])
            return x_sb

        qt_g = kt_g = vp_g = wo_sb = None
        for _rep in range(reps):
            do_proj = _rep == 0 or rep_phase in ("all", "proj")
            do_attn = _rep == 0 or rep_phase in ("all", "attn")
            do_out = _rep == 0 or rep_phase in ("all", "out")
            if do_proj:
                qt_g = gpool.tile([128, FCG * R], MMDT, tag="qt")
                kt_g = gpool.tile([128, FCG * S], MMDT, tag="kt")
                vp_g = gpool.tile([128, NKC * WV], VPDT, tag="vp")
                # fill the per-head ones columns (col 64 of each 65-wide slot)
                nc.vector.tensor_copy(
                    vp_g[:].rearrange("p (a e) -> p a e", e=65)[:, :, 64:65],
                    ones_wide[:].unsqueeze(2),
                )

            with tc.tile_pool(name="pp", bufs=3, space="PSUM") as ppool:
                if do_proj:
                    # ---- Q projection ----
                    w_sb = load_w(wqT, ppool)
                    for rb in range(R // XW):
                        x_sb = load_x(qT, rb, ppool)
                        for fcg in range(FCG):
                            ps = ppool.tile([128, XW], F32, tag="ps")
                            for ic in range(IC):
                                nc.tensor.matmul(
                                    ps[:],
                                    w_sb[:, ic * FG + fcg * 128:
                                         ic * FG + fcg * 128 + 128],
                                    x_sb[:, ic * XW:(ic + 1) * XW],
                                    start=(ic == 0),
                                    stop=False,
                                )
                            nc.tensor.matmul(
                                ps[:],
                                bq_sb[0:1, fcg * 128:(fcg + 1) * 128],
                                mrow[0:1, rb * XW:(rb + 1) * XW],
                                start=False,
                                stop=True,
                            )
                            nc.vector.tensor_copy(
                                qt_g[:, fcg * R + rb * XW:
                                     fcg * R + (rb + 1) * XW],
                                ps[:],
                            )

                    # ---- K projection ----
                    w_sb = load_w(wkT, ppool)
                    for rb in range(S // XW):
                        x_sb = load_x(kT, rb, ppool)
                        for fcg in range(FCG):
                            ps = ppool.tile([128, XW], F32, tag="ps")
                            for ic in range(IC):
                                nc.tensor.matmul(
                                    ps[:],
                                    w_sb[:, ic * FG + fcg * 128:
                                         ic * FG + fcg * 128 + 128],
                                    x_sb[:, ic * XW:(ic + 1) * XW],
                                    start=(ic == 0),
                                    stop=False,
                                )
                            nc.tensor.matmul(
                                ps[:],
                                bk_sb[0:1, fcg * 128:(fcg + 1) * 128],
                                ones_row[0:1, 0:XW],
                                start=False,
                                stop=True,
                            )
                            nc.vector.tensor_copy(
                                kt_g[:, fcg * S + rb * XW:
                                     fcg * S + (rb + 1) * XW],
                                ps[:],
                            )

                    # ---- V projection + ones column ----
                    w_sb = load_w(wvT, ppool)
                    NRC = XW // 128
                    for rb4 in range(S // XW):
                        x_sb = load_x(vT, rb4, ppool)
                        for rcl in range(NRC):
                            rc = rb4 * NRC + rcl
                            ps = ppool.tile([128, FG], F32, tag="ps")
                            for ic in range(IC):
                                nc.tensor.matmul(
                                    ps[:],
                                    x_sb[:, ic * XW + rcl * 128:
                                         ic * XW + rcl * 128 + 128],
                                    w_sb[:, ic * FG:(ic + 1) * FG],
                                    start=(ic == 0),
                                    stop=False,
                                )
                            nc.tensor.matmul(
                                ps[:],
                                ones_row[0:1, 0:128],
                                bv_sb[0:1, 0:FG],
                                start=False,
                                stop=True,
                            )
                            for hl in range(HPG):
                                nc.vector.tensor_copy(
                                    vp_g[:, rc * WV + 65 * hl:
                                         rc * WV + 65 * hl + 64],
                                    ps[:, hl * 64:(hl + 1) * 64],
                                )

            if do_proj:
                # wo load here overlaps attention; needed only at out proj
                wo_sb = wopool.tile([128, FCG * HID], MMDT, tag="wos")
                nc.sync.dma_start(
                    wo_sb[:].rearrange("p (f h) -> p f h", f=FCG),
                    woT[:, :].rearrange("(f p) h -> p f h", p=128),
                )

            # ---- attention: head pairs share the PE array via row groups ----
            with tc.tile_pool(name="sps", bufs=3, space="PSUM") as spool, \
                 tc.tile_pool(name="pvp", bufs=2, space="PSUM") as pvpool, \
                 tc.tile_pool(name="esb", bufs=2 if expmode == "f32r" else 3) as epool, \
                 tc.tile_pool(name="nrm", bufs=2) as npool, \
                 tc.tile_pool(name="pvs", bufs=2) as pvspool:
                for sqb in range(NSQB if do_attn else 0):
                    for hp in range(HPG // 2):
                        fcg = hp
                        q0 = qt_g[0:64,
                                  fcg * R + sqb * SQB: fcg * R + (sqb + 1) * SQB]
                        q1 = qt_g[64:128,
                                  fcg * R + sqb * SQB: fcg * R + (sqb + 1) * SQB]
                        pv0 = pvpool.tile([65, SQB], F32, tag="pv")
                        pv1 = pvpool.tile([65, SQB], F32, tag="pv")
                        pvs = [pv0, pv1]
                        for kch in range(NKC // 2):
                            sp0 = spool.tile([128, 2 * SQB], F32, tag="sp")
                            sp1 = spool.tile([128, 2 * SQB], F32, tag="sp")
                            sps = [sp0, sp1]
                            for j in range(2):
                                kc = 2 * kch + j
                                kslc = slice(fcg * S + kc * 128,
                                             fcg * S + kc * 128 + 128)
                                # heads 2hp (rows 0-63) and 2hp+1 (rows 64-127)
                                # run concurrently in disjoint PE row groups
                                nc.tensor.matmul(
                                    sps[0][:, j * SQB:(j + 1) * SQB],
                                    kt_g[0:64, kslc], q0, start=True, stop=True,
                                )
                                nc.tensor.matmul(
                                    sps[1][:, j * SQB:(j + 1) * SQB],
                                    kt_g[64:128, kslc], q1, start=True, stop=True,
                                )
                            ess = []
                            for h in range(2):
                                es = epool.tile([128, 2 * SQB], ESDT, tag="es")
                                nc.scalar.activation(es[:], sps[h][:], AF.Exp)
                                ess.append(es)
                            for j in range(2):
                                kc = 2 * kch + j
                                for h in range(2):
                                    hl = 2 * hp + h
                                    nc.tensor.matmul(
                                        pvs[h][:],
                                        vp_g[:, kc * WV + 65 * hl:
                                             kc * WV + 65 * hl + 65],
                                        ess[h][:, j * SQB:(j + 1) * SQB],
                                        start=(kc == 0),
                                        stop=(kc == NKC - 1),
                                    )
                        for h in range(2):
                            po = 64 * h
                            # copy PSUM->SBUF immediately to free the bank,
                            # then normalize off the SBUF copy
                            pv_sb = pvspool.tile([65, SQB], F32, tag="pvsb")
                            nc.vector.tensor_copy(pv_sb[:], pvs[h][:])
                            recip = npool.tile([1, SQB], F32, tag="recip")
                            nc.vector.reciprocal(recip[:], pv_sb[64:65, :])
                            rd = dbounce.tile([1, SQB], F32, tag="rd")
                            nc.sync.dma_start(rd[:], recip[:])
                            recipB = npool.tile([64, SQB], F32, tag="recipB")
                            nc.sync.dma_start(
                                recipB[:], rd[:].to_broadcast([64, SQB])
                            )
                            nc.vector.tensor_copy(
                                recipB[0:1, 0:1], recipB[0:1, 0:1]
                            )
                            nc.vector.tensor_mul(
                                h_tile[po:po + 64, fcg * R + sqb * SQB:
                                       fcg * R + (sqb + 1) * SQB],
                                pv_sb[0:64, :],
                                recipB[:],
                            )

            # ---- output projection: partial over this core's hidden dims ----
            OB = 512
            NOB = HID // OB       # 2
            with tc.tile_pool(name="ops", bufs=8, space="PSUM") as opool, \
                 tc.tile_pool(name="osb", bufs=4) as ospool:
                for quarter in range(R // 128 // 4 if do_out else 0):
                    pss = []
                    for _psi in range(4 * NOB):
                        ps_acc = opool.tile([128, OB], F32, tag="ps")
                        pss.append(ps_acc)
                    for fc in range(FCG):
                        for rl in range(4):
                            rc = quarter * 4 + rl
                            for ob in range(NOB):
                                nc.tensor.matmul(
                                    pss[rl * NOB + ob][:],
                                    h_tile[:, fc * R + rc * 128:
                                           fc * R + rc * 128 + 128],
                                    wo_sb[:, fc * HID + ob * OB:
                                          fc * HID + (ob + 1) * OB],
                                    start=(fc == 0),
                                    stop=(fc == FCG - 1),
                                )
                    for rl in range(4):
                        rc = quarter * 4 + rl
                        for ob in range(NOB):
                            o_sb = ospool.tile([128, OB], F32, tag="o")
                            nc.vector.tensor_copy(o_sb[:], pss[rl * NOB + ob][:])
                            nc.sync.dma_start(
                                out[rc * 128:(rc + 1) * 128,
                                    ob * OB:(ob + 1) * OB],
                                o_sb[:],
                            )
    return nc



def build4(nc: bass.Bass, cfg: Cfg3, reps: int = 1, expmode: str = "f32r"):
    """Interleaved schedule: KV proj first, then per-query-block loop where
    Q projection (next block) and output projection (previous block) are
    emitted between attention head-pair blocks so PE fills the gaps while
    the Activation engine streams exp. expmode "f32r": exp writes f32 and
    PV matmuls run f32r (avoids the slow PSUM->bf16 activation path);
    "bf16": exp writes bf16, PV in bf16."""
    HID, R, S = cfg.HID, cfg.R, cfg.S
    IC, HPG, FG, FCG = cfg.IC, cfg.HPG, cfg.FG, cfg.FCG
    NKC, SQB, NSQB, WV, XW = cfg.NKC, cfg.SQB, cfg.NSQB, cfg.WV, cfg.XW
    MMDT = cfg.MMDT
    F32R = DT.float32r
    ESDT = F32R if expmode == "f32r" else MMDT
    VPDT = F32R if expmode == "f32r" else MMDT

    dp = nc.declare_dram_parameter
    qT = dp("qT", [HID, R], MMDT, isOutput=False)
    kT = dp("kT", [HID, S], MMDT, isOutput=False)
    vT = dp("vT", [HID, S], MMDT, isOutput=False)
    wqT = dp("wqT", [HID, FG], MMDT, isOutput=False)
    wkT = dp("wkT", [HID, FG], MMDT, isOutput=False)
    wvT = dp("wvT", [HID, FG], MMDT, isOutput=False)
    woT = dp("woT", [FG, HID], MMDT, isOutput=False)
    bqr = dp("bqr", [1, FG], MMDT, isOutput=False)
    bkr = dp("bkr", [1, FG], MMDT, isOutput=False)
    bvr = dp("bvr", [1, FG], MMDT, isOutput=False)
    maskf = dp("maskf", [1, R], F32, isOutput=False)
    out = dp("out", [R, HID], F32, isOutput=True)

    def pv_ops(a, b):
        return a, b

    with tile.TileContext(nc) as tc, ExitStack() as ctx:
        cpool = ctx.enter_context(tc.tile_pool(name="consts", bufs=1))
        bq_sb = cpool.tile([1, FG], MMDT, tag="bq")
        bk_sb = cpool.tile([1, FG], MMDT, tag="bk")
        bv_sb = cpool.tile([1, FG], MMDT, tag="bv")
        nc.sync.dma_start(bq_sb[:], bqr[:])
        nc.sync.dma_start(bk_sb[:], bkr[:])
        nc.sync.dma_start(bv_sb[:], bvr[:])
        ones_f32 = cpool.tile([1, XW], F32, tag="ones32")
        nc.vector.memset(ones_f32[:], 1.0)
        ones_row = cpool.tile([1, XW], MMDT, tag="ones")
        nc.vector.tensor_copy(ones_row[:], ones_f32[:])
        NOC = NKC * HPG
        onesw_f32 = cpool.tile([128, NOC], F32, tag="onesw32")
        nc.vector.memset(onesw_f32[:], 1.0)
        ones_wide = cpool.tile([128, NOC], VPDT, tag="onesw")
        nc.vector.tensor_copy(ones_wide[:], onesw_f32[:])
        mrow_f32 = cpool.tile([1, R], F32, tag="mrow32")
        nc.sync.dma_start(mrow_f32[:], maskf[:])
        mrow = cpool.tile([1, R], MMDT, tag="mrow")
        nc.vector.tensor_copy(mrow[:], mrow_f32[:])
        dbounce = ctx.enter_context(
            tc.tile_pool(name="dbounce", bufs=4, space="DRAM"))

        gpool = ctx.enter_context(tc.tile_pool(name="gstore", bufs=1))
        h_tile = gpool.tile([128, FCG * R], MMDT, tag="h")

        wpool = ctx.enter_context(tc.tile_pool(name="wgt", bufs=2))
        xpool = ctx.enter_context(tc.tile_pool(name="xin", bufs=2))
        wqpool = ctx.enter_context(tc.tile_pool(name="wq", bufs=1))
        xqpool = ctx.enter_context(tc.tile_pool(name="xq", bufs=1))
        wopool = ctx.enter_context(tc.tile_pool(name="wo", bufs=1))

        def pe_touch(ppool, ap):
            pt = ppool.tile([1, 1], F32, tag="pt", bufs=2)
            nc.tensor.matmul(pt[:], ap, ap, start=True, stop=True)

        def load_w(wT, ppool):
            w_sb = wpool.tile([128, IC * FG], MMDT, tag="w")
            src = wT[:, :].rearrange("(i p) f -> p i f", p=128)
            nc.sync.dma_start(w_sb[:].rearrange("p (i f) -> p i f", i=IC), src)
            pe_touch(ppool, w_sb[0:1, 0:1])
            return w_sb

        def load_x(xT, rb, ppool):
            x_sb = xpool.tile([128, IC * XW], MMDT, tag="x")
            src = xT[:, rb * XW:(rb + 1) * XW].rearrange(
                "(i p) w -> p i w", p=128)
            nc.sync.dma_start(x_sb[:].rearrange("p (i w) -> p i w", i=IC), src)
            pe_touch(ppool, x_sb[0:1, 0:1])
            return x_sb

        for _rep in range(reps):
            qt_g = gpool.tile([128, FCG * R], MMDT, tag="qt")
            kt_g = gpool.tile([128, FCG * S], MMDT, tag="kt")
            vp_g = gpool.tile([128, NKC * WV], VPDT, tag="vp")
            nc.vector.tensor_copy(
                vp_g[:].rearrange("p (a e) -> p a e", e=65)[:, :, 64:65],
                ones_wide[:].unsqueeze(2),
            )

            with tc.tile_pool(name="pp", bufs=3, space="PSUM") as ppool:
                # ---- K projection ----
                w_sb = load_w(wkT, ppool)
                for rb in range(S // XW):
                    x_sb = load_x(kT, rb, ppool)
                    for fcg in range(FCG):
                        ps = ppool.tile([128, XW], F32, tag="ps")
                        for ic in range(IC):
                            nc.tensor.matmul(
                                ps[:],
                                w_sb[:, ic * FG + fcg * 128:
                                     ic * FG + fcg * 128 + 128],
                                x_sb[:, ic * XW:(ic + 1) * XW],
                                start=(ic == 0),
                                stop=False,
                            )
                        nc.tensor.matmul(
                            ps[:],
                            bk_sb[0:1, fcg * 128:(fcg + 1) * 128],
                            ones_row[0:1, 0:XW],
                            start=False,
                            stop=True,
                        )
                        nc.vector.tensor_copy(
                            kt_g[:, fcg * S + rb * XW:
                                 fcg * S + (rb + 1) * XW],
                            ps[:],
                        )

                # ---- V projection + ones column ----
                w_sb = load_w(wvT, ppool)
                NRC = XW // 128
                for rb4 in range(S // XW):
                    x_sb = load_x(vT, rb4, ppool)
                    for rcl in range(NRC):
                        rc = rb4 * NRC + rcl
                        ps = ppool.tile([128, FG], F32, tag="ps")
                        for ic in range(IC):
                            nc.tensor.matmul(
                                ps[:],
                                x_sb[:, ic * XW + rcl * 128:
                                     ic * XW + rcl * 128 + 128],
                                w_sb[:, ic * FG:(ic + 1) * FG],
                                start=(ic == 0),
                                stop=False,
                            )
                        nc.tensor.matmul(
                            ps[:],
                            ones_row[0:1, 0:128],
                            bv_sb[0:1, 0:FG],
                            start=False,
                            stop=True,
                        )
                        for hl in range(HPG):
                            nc.vector.tensor_copy(
                                vp_g[:, rc * WV + 65 * hl:
                                     rc * WV + 65 * hl + 64],
                                ps[:, hl * 64:(hl + 1) * 64],
                            )

                # ---- resident Q weights + all Q input blocks + wo ----
                wq_sb = wqpool.tile([128, IC * FG], MMDT, tag="wqs")
                nc.sync.dma_start(
                    wq_sb[:].rearrange("p (i f) -> p i f", i=IC),
                    wqT[:, :].rearrange("(i p) f -> p i f", p=128),
                )
                pe_touch(ppool, wq_sb[0:1, 0:1])
                xq_all = xqpool.tile([128, IC * R], MMDT, tag="xqa")
                nc.sync.dma_start(
                    xq_all[:].rearrange("p (i w) -> p i w", i=IC),
                    qT[:, :].rearrange("(i p) w -> p i w", p=128),
                )
                pe_touch(ppool, xq_all[0:1, 0:1])
                wo_sb = wopool.tile([128, FCG * HID], MMDT, tag="wos")
                nc.sync.dma_start(
                    wo_sb[:].rearrange("p (f h) -> p f h", f=FCG),
                    woT[:, :].rearrange("(f p) h -> p f h", p=128),
                )

                # ---- Q projection for sqb 0 (rest interleaved below) ----
                def qproj_block(pool, sqb, fcgs):
                    for fcg in fcgs:
                        ps = pool.tile([128, SQB], F32, tag="qo")
                        for ic in range(IC):
                            nc.tensor.matmul(
                                ps[:],
                                wq_sb[:, ic * FG + fcg * 128:
                                      ic * FG + fcg * 128 + 128],
                                xq_all[:, ic * R + sqb * SQB:
                                       ic * R + (sqb + 1) * SQB],
                                start=(ic == 0),
                                stop=False,
                            )
                        nc.tensor.matmul(
                            ps[:],
                            bq_sb[0:1, fcg * 128:(fcg + 1) * 128],
                            mrow[0:1, sqb * SQB:(sqb + 1) * SQB],
                            start=False,
                            stop=True,
                        )
                        nc.vector.tensor_copy(
                            qt_g[:, fcg * R + sqb * SQB:
                                 fcg * R + (sqb + 1) * SQB],
                            ps[:],
                        )

                qproj_block(ppool, 0, range(FCG))

            # ---- attention + interleaved Q proj / out proj ----
            OB = 512
            NOB = HID // OB

            with tc.tile_pool(name="sps", bufs=2, space="PSUM") as spool, \
                 tc.tile_pool(name="pvp", bufs=2, space="PSUM") as pvpool, \
                 tc.tile_pool(name="sml", bufs=2, space="PSUM") as smpool, \
                 tc.tile_pool(name="esb", bufs=2) as epool, \
                 tc.tile_pool(name="nrm", bufs=2) as npool, \
                 tc.tile_pool(name="osb", bufs=2) as ospool, \
                 tc.tile_pool(name="pvs", bufs=2) as pvspool:

                def out_block(sqb, ob):
                    for rl in range(SQB // 128):
                        rc = sqb * (SQB // 128) + rl
                        ps = smpool.tile([128, OB], F32, tag="qo")
                        for fc in range(FCG):
                            nc.tensor.matmul(
                                ps[:],
                                h_tile[:, fc * R + rc * 128:
                                       fc * R + rc * 128 + 128],
                                wo_sb[:, fc * HID + ob * OB:
                                      fc * HID + (ob + 1) * OB],
                                start=(fc == 0),
                                stop=(fc == FCG - 1),
                            )
                        o_sb = ospool.tile([128, OB], F32, tag="o")
                        nc.vector.tensor_copy(o_sb[:], ps[:])
                        nc.sync.dma_start(
                            out[rc * 128:(rc + 1) * 128,
                                ob * OB:(ob + 1) * OB],
                            o_sb[:],
                        )

                for sqb in range(NSQB):
                    for hp in range(HPG // 2):
                        fcg = hp
                        q0 = qt_g[0:64,
                                  fcg * R + sqb * SQB: fcg * R + (sqb + 1) * SQB]
                        q1 = qt_g[64:128,
                                  fcg * R + sqb * SQB: fcg * R + (sqb + 1) * SQB]
                        pv0 = pvpool.tile([65, SQB], F32, tag="pv")
                        pv1 = pvpool.tile([65, SQB], F32, tag="pv")
                        pvs = [pv0, pv1]
                        for kch in range(NKC // 2):
                            sp0 = spool.tile([128, 2 * SQB], F32, tag="sp")
                            sp1 = spool.tile([128, 2 * SQB], F32, tag="sp")
                            sps = [sp0, sp1]
                            for j in range(2):
                                kc = 2 * kch + j
                                kslc = slice(fcg * S + kc * 128,
                                             fcg * S + kc * 128 + 128)
                                nc.tensor.matmul(
                                    sps[0][:, j * SQB:(j + 1) * SQB],
                                    kt_g[0:64, kslc], q0, start=True, stop=True,
                                )
                                nc.tensor.matmul(
                                    sps[1][:, j * SQB:(j + 1) * SQB],
                                    kt_g[64:128, kslc], q1, start=True, stop=True,
                                )
                            ess = []
                            for h in range(2):
                                es = epool.tile([128, 2 * SQB], ESDT, tag="es")
                                nc.scalar.activation(es[:], sps[h][:], AF.Exp)
                                ess.append(es)
                            for j in range(2):
                                kc = 2 * kch + j
                                for h in range(2):
                                    hl = 2 * hp + h
                                    a_op, b_op = pv_ops(
                                        vp_g[:, kc * WV + 65 * hl:
                                             kc * WV + 65 * hl + 65],
                                        ess[h][:, j * SQB:(j + 1) * SQB],
                                    )
                                    nc.tensor.matmul(
                                        pvs[h][:],
                                        a_op,
                                        b_op,
                                        start=(kc == 0),
                                        stop=(kc == NKC - 1),
                                    )
                        for h in range(2):
                            po = 64 * h
                            pv_sb = pvspool.tile([65, SQB], F32, tag="pvsb")
                            nc.vector.tensor_copy(pv_sb[:], pvs[h][:])
                            recip = npool.tile([1, SQB], F32, tag="recip")
                            nc.vector.reciprocal(recip[:], pv_sb[64:65, :])
                            rd = dbounce.tile([1, SQB], F32, tag="rd")
                            nc.sync.dma_start(rd[:], recip[:])
                            recipB = npool.tile([64, SQB], F32, tag="recipB")
                            nc.sync.dma_start(
                                recipB[:], rd[:].to_broadcast([64, SQB])
                            )
                            nc.vector.tensor_copy(
                                recipB[0:1, 0:1], recipB[0:1, 0:1]
                            )
                            nc.vector.tensor_mul(
                                h_tile[po:po + 64, fcg * R + sqb * SQB:
                                       fcg * R + (sqb + 1) * SQB],
                                pv_sb[0:64, :],
                                recipB[:],
                            )
                        # interleaved PE filler: next Q block, prev out rows
                        if hp == 0 and sqb + 1 < NSQB:
                            qproj_block(smpool, sqb + 1, (0, 1))
                        elif hp == 1 and sqb + 1 < NSQB:
                            qproj_block(smpool, sqb + 1, (2, 3))
                        elif hp == 2 and sqb >= 1:
                            out_block(sqb - 1, 0)
                        elif hp == 3 and sqb >= 1:
                            out_block(sqb - 1, 1)

                out_block(NSQB - 1, 0)
                out_block(NSQB - 1, 1)
    return nc


_compiled = {}


SCHED = "v3"      # "v3" | "v4"
EXPMODE = "f32r"  # for v4: "f32r" | "bf16"


def _get_nc(cfg_key):
    if cfg_key not in _compiled:
        cfg = Cfg3(HID=HID, NH=NH, S=S, use_bf16=USE_BF16, use_f32r=USE_F32R)
        nc = bacc.Bacc(
            "TRN2", target_bir_lowering=False, debug=False, num_devices=N_CORES
        )
        if SCHED == "v4":
            build4(nc, cfg, expmode=EXPMODE)
        else:
            build3(nc, cfg, expmode=EXPMODE)
        nc.compile()
        _compiled[cfg_key] = (nc, cfg)
    return _compiled[cfg_key]


def make_in_maps(cfg, q, k, v, mask, wq, bq, wk, bk, wv, bv, wo, bo):
    """Per-core input maps for head-parallel sharding."""
    f32 = np.float32
    mmnp = mybir.dt.np(cfg.MMDT) if USE_BF16 else f32
    scale = f32(1.0 / np.sqrt(DH))
    FG = cfg.FG
    mrow_b = [((np.asarray(mask[b]) != 0).astype(f32) * scale)
              for b in range(B)]
    qT_b = [(np.asarray(q[b], f32) * mrow_b[b][:, None]).T.astype(mmnp)
            for b in range(B)]
    kT_b = [np.asarray(k[b], f32).T.astype(mmnp) for b in range(B)]
    vT_b = [np.asarray(v[b], f32).T.astype(mmnp) for b in range(B)]
    mask_b = [mrow_b[b].reshape(1, S) for b in range(B)]
    wqT = np.asarray(wq, f32).T
    wkT = np.asarray(wk, f32).T
    wvT = np.asarray(wv, f32).T
    woT = np.asarray(wo, f32).T
    in_maps = []
    for c in range(N_CORES):
        b, hg = c // 2, c % 2
        hs = slice(hg * FG, (hg + 1) * FG)
        m = {
            "qT": qT_b[b],
            "kT": kT_b[b],
            "vT": vT_b[b],
            "maskf": mask_b[b],
            "wqT": wqT[:, hs].astype(mmnp),
            "wkT": wkT[:, hs].astype(mmnp),
            "wvT": wvT[:, hs].astype(mmnp),
            "woT": woT[hs, :].astype(mmnp),
            "bqr": (np.asarray(bq, f32)[hs] * scale
                    ).reshape(1, FG).astype(mmnp),
            "bkr": np.asarray(bk, f32)[hs].reshape(1, FG).astype(mmnp),
            "bvr": np.asarray(bv, f32)[hs].reshape(1, FG).astype(mmnp),
        }
        in_maps.append(m)
    return in_maps


def kernel(q, k, v, mask, wq, bq, wk, bk, wv, bv, wo, bo):
    q = np.asarray(q, dtype=np.float32)
    k = np.asarray(k, dtype=np.float32)
    v = np.asarray(v, dtype=np.float32)
    mask = np.asarray(mask)
    f32 = np.float32

    nc, cfg = _get_nc((USE_F32R, USE_BF16, SHARD, SCHED, EXPMODE))
    in_maps = make_in_maps(cfg, q, k, v, mask, wq, bq, wk, bk, wv, bv, wo, bo)

    res = run_bass_kernel_spmd(nc, in_maps, list(range(N_CORES)), trace=TRACE)
    LAST_RESULTS[0] = res

    out = np.empty((B, S, HID), dtype=np.float32)
    bof = np.asarray(bo, f32).reshape(1, HID)
    for b in range(B):
        out[b] = res.results[2 * b]["out"]
        out[b] += res.results[2 * b + 1]["out"]
        out[b] += bof
    return out


# revision 16
# speedup vs baseline: 1.7683x; 1.7683x over previous
"""TRN2 Bass kernel: 16-head attention (B=4, S=2048, HID=1024), fp32 I/O.

Full inputs in, full output out. Head-parallel sharding across 8 cores:
core c handles batch c//2 and heads [8*(c%2), 8*(c%2)+8) — Q/K/V
projections only for those 8 heads (512 of 1024 hidden dims), attention
over the full sequence, and a partial output projection over its 512
hidden dims. The host sums the two partials per batch and adds bo
(row-parallel tensor parallelism; no device collectives).

Matmuls run in bf16 (PSUM accumulates fp32); rel-err stays ~4e-3 vs the
fp32 reference, dominated by input quantization. Mask semantics: the
reference masks QUERY rows with -1e9 before softmax, which yields a
uniform distribution for masked rows. Here masked q rows are zeroed on
the host (scores 0 -> uniform softmax), and the Q bias is applied
through the mask row so bq on masked rows stays zero.

Device pipeline per core (8 heads, dh=64):
  QT[f,r] = wqT.T @ qT_scaled + bq*s*m   (host pre-scales q by s*mask)
  KT[f,k] = wkT.T @ kT + bk
  V'[k,f] = vT.T @ wvT + bv, with a ones column per head
  scoresT[k,sq] = KT_h.T @ QT_h  -> exp on ScalarE -> PV psum += V'_h.T @ expS
  (PV row 64 = softmax denominator via the ones column.)
  H = PV[0:64] * (1/denom);  out_partial[r,:] = H.T @ woT_half
"""

from contextlib import ExitStack

import numpy as np

import concourse.bass as bass
import concourse.bacc as bacc
import concourse.mybir as mybir
import concourse.tile as tile
from concourse.bass_utils import run_bass_kernel_spmd

DT = mybir.dt
F32 = DT.float32
AF = mybir.ActivationFunctionType
ALU = mybir.AluOpType

# Problem constants (hardcoded per harness contract)
B, S, HID, NH, DH = 4, 2048, 1024, 16, 64
N_CORES = 8

# toggles (kept as globals so test.py can flip them)
USE_F32R = True
USE_BF16 = True  # wins over USE_F32R when set
SHARD = "head"
TRACE = False
LAST_RESULTS = [None]


class Cfg3:
    """Head-parallel sharding: core = (batch, head-group of 8)."""

    def __init__(self, HID=1024, NH=16, S=2048, use_bf16=True, use_f32r=True):
        self.HID, self.NH, self.S = HID, NH, S
        self.R = S              # full batch queries per core
        self.DH = 64
        self.HPG = NH // 2      # 8 heads per core
        self.FG = self.HPG * self.DH          # 512 hidden dims per core
        self.FCG = self.FG // 128             # 4 head-pair chunks
        self.IC = HID // 128                  # 8 contraction chunks
        self.NKC = S // 128                   # 16 key chunks
        self.SQB = 512
        self.NSQB = self.R // self.SQB        # 4
        self.WV = self.HPG * 65               # 520
        self.XW = 512
        if use_bf16:
            self.MMDT = DT.bfloat16
        else:
            self.MMDT = DT.float32r if use_f32r else DT.float32


def build3(nc: bass.Bass, cfg: Cfg3, reps: int = 1, rep_phase: str = "all",
           expmode: str = "bf16"):
    HID, R, S = cfg.HID, cfg.R, cfg.S
    IC, HPG, FG, FCG = cfg.IC, cfg.HPG, cfg.FG, cfg.FCG
    NKC, SQB, NSQB, WV, XW = cfg.NKC, cfg.SQB, cfg.NSQB, cfg.WV, cfg.XW
    MMDT = cfg.MMDT
    ESDT = DT.float32r if expmode == "f32r" else MMDT
    VPDT = DT.float32r if expmode == "f32r" else MMDT

    dp = nc.declare_dram_parameter
    qT = dp("qT", [HID, R], MMDT, isOutput=False)
    kT = dp("kT", [HID, S], MMDT, isOutput=False)
    vT = dp("vT", [HID, S], MMDT, isOutput=False)
    wqT = dp("wqT", [HID, FG], MMDT, isOutput=False)
    wkT = dp("wkT", [HID, FG], MMDT, isOutput=False)
    wvT = dp("wvT", [HID, FG], MMDT, isOutput=False)
    woT = dp("woT", [FG, HID], MMDT, isOutput=False)
    bqr = dp("bqr", [1, FG], MMDT, isOutput=False)
    bkr = dp("bkr", [1, FG], MMDT, isOutput=False)
    bvr = dp("bvr", [1, FG], MMDT, isOutput=False)
    maskf = dp("maskf", [1, R], F32, isOutput=False)
    out = dp("out", [R, HID], F32, isOutput=True)  # partial: host adds pair+bo

    with tile.TileContext(nc) as tc, ExitStack() as ctx:
        cpool = ctx.enter_context(tc.tile_pool(name="consts", bufs=1))
        bq_sb = cpool.tile([1, FG], MMDT, tag="bq")
        bk_sb = cpool.tile([1, FG], MMDT, tag="bk")
        bv_sb = cpool.tile([1, FG], MMDT, tag="bv")
        nc.sync.dma_start(bq_sb[:], bqr[:])
        nc.sync.dma_start(bk_sb[:], bkr[:])
        nc.sync.dma_start(bv_sb[:], bvr[:])
        ones_f32 = cpool.tile([1, XW], F32, tag="ones32")
        nc.vector.memset(ones_f32[:], 1.0)
        ones_row = cpool.tile([1, XW], MMDT, tag="ones")
        nc.vector.tensor_copy(ones_row[:], ones_f32[:])
        NOC = NKC * HPG  # ones-column count in V' (128)
        onesw_f32 = cpool.tile([128, NOC], F32, tag="onesw32")
        nc.vector.memset(onesw_f32[:], 1.0)
        ones_wide = cpool.tile([128, NOC], VPDT, tag="onesw")
        nc.vector.tensor_copy(ones_wide[:], onesw_f32[:])
        mrow_f32 = cpool.tile([1, R], F32, tag="mrow32")
        nc.sync.dma_start(mrow_f32[:], maskf[:])
        mrow = cpool.tile([1, R], MMDT, tag="mrow")
        nc.vector.tensor_copy(mrow[:], mrow_f32[:])
        dbounce = ctx.enter_context(
            tc.tile_pool(name="dbounce", bufs=4, space="DRAM"))

        gpool = ctx.enter_context(tc.tile_pool(name="gstore", bufs=1))
        h_tile = gpool.tile([128, FCG * R], MMDT, tag="h")

        # persistent pools so weight/activation DMAs prefetch across phases
        wpool = ctx.enter_context(tc.tile_pool(name="wgt", bufs=2))
        xpool = ctx.enter_context(tc.tile_pool(name="xin", bufs=2))
        wopool = ctx.enter_context(tc.tile_pool(name="wo", bufs=1))

        def pe_touch(ppool, ap):
            # 1x1 matmul that absorbs a DMA-queue wait into the PE clock, so
            # real matmuls stay within the 2-sync-wait ISA budget
            pt = ppool.tile([1, 1], F32, tag="pt", bufs=2)
            if MMDT == DT.float32r:
                ap = ap.bitcast(F32)
            nc.tensor.matmul(pt[:], ap, ap, start=True, stop=True)

        def load_w(wT, ppool):
            w_sb = wpool.tile([128, IC * FG], MMDT, tag="w")
            src = wT[:, :].rearrange("(i p) f -> p i f", p=128)
            nc.sync.dma_start(w_sb[:].rearrange("p (i f) -> p i f", i=IC), src)
            pe_touch(ppool, w_sb[0:1, 0:1])
            return w_sb

        def load_x(xT, rb, ppool):
            x_sb = xpool.tile([128, IC * XW], MMDT, tag="x")
            src = xT[:, rb * XW:(rb + 1) * XW].rearrange(
                "(i p) w -> p i w", p=128)
            nc.sync.dma_start(x_sb[:].rearrange("p (i w) -> p i w", i=IC), src)
            pe_touch(ppool, x_sb[0:1, 0:1])
            return x_sb

        qt_g = kt_g = vp_g = wo_sb = None
        for _rep in range(reps):
            do_proj = _rep == 0 or rep_phase in ("all", "proj")
            do_attn = _rep == 0 or rep_phase in ("all", "attn")
            do_out = _rep == 0 or rep_phase in ("all", "out")
            if do_proj:
                qt_g = gpool.tile([128, FCG * R], MMDT, tag="qt")
                kt_g = gpool.tile([128, FCG * S], MMDT, tag="kt")
                vp_g = gpool.tile([128, NKC * WV], VPDT, tag="vp")
                # fill the per-head ones columns (col 64 of each 65-wide slot)
                nc.vector.tensor_copy(
                    vp_g[:].rearrange("p (a e) -> p a e", e=65)[:, :, 64:65],
                    ones_wide[:].unsqueeze(2),
                )

            with tc.tile_pool(name="pp", bufs=3, space="PSUM") as ppool:
                if do_proj:
                    # ---- Q projection ----
                    w_sb = load_w(wqT, ppool)
                    for rb in range(R // XW):
                        x_sb = load_x(qT, rb, ppool)
                        for fcg in range(FCG):
                            ps = ppool.tile([128, XW], F32, tag="ps")
                            for ic in range(IC):
                                nc.tensor.matmul(
                                    ps[:],
                                    w_sb[:, ic * FG + fcg * 128:
                                         ic * FG + fcg * 128 + 128],
                                    x_sb[:, ic * XW:(ic + 1) * XW],
                                    start=(ic == 0),
                                    stop=False,
                                )
                            nc.tensor.matmul(
                                ps[:],
                                bq_sb[0:1, fcg * 128:(fcg + 1) * 128],
                                mrow[0:1, rb * XW:(rb + 1) * XW],
                                start=False,
                                stop=True,
                            )
                            nc.vector.tensor_copy(
                                qt_g[:, fcg * R + rb * XW:
                                     fcg * R + (rb + 1) * XW],
                                ps[:],
                            )

                    # ---- K projection ----
                    w_sb = load_w(wkT, ppool)
                    for rb in range(S // XW):
                        x_sb = load_x(kT, rb, ppool)
                        for fcg in range(FCG):
                            ps = ppool.tile([128, XW], F32, tag="ps")
                            for ic in range(IC):
                                nc.tensor.matmul(
                                    ps[:],
                                    w_sb[:, ic * FG + fcg * 128:
                                         ic * FG + fcg * 128 + 128],
                                    x_sb[:, ic * XW:(ic + 1) * XW],
                                    start=(ic == 0),
                                    stop=False,
                                )
                            nc.tensor.matmul(
                                ps[:],
                                bk_sb[0:1, fcg * 128:(fcg + 1) * 128],
                                ones_row[0:1, 0:XW],
                                start=False,
                                stop=True,
                            )
                            nc.vector.tensor_copy(
                                kt_g[:, fcg * S + rb * XW:
                                     fcg * S + (rb + 1) * XW],
                                ps[:],
                            )

                    # ---- V projection + ones column ----
                    w_sb = load_w(wvT, ppool)
                    NRC = XW // 128
                    for rb4 in range(S // XW):
                        x_sb = load_x(vT, rb4, ppool)
                        for rcl in range(NRC):
                            rc = rb4 * NRC + rcl
                            ps = ppool.tile([128, FG], F32, tag="ps")
                            for ic in range(IC):
                                nc.tensor.matmul(
                                    ps[:],
                                    x_sb[:, ic * XW + rcl * 128:
                                         ic * XW + rcl * 128 + 128],
                                    w_sb[:, ic * FG:(ic + 1) * FG],
                                    start=(ic == 0),
                                    stop=False,
                                )
                            nc.tensor.matmul(
                                ps[:],
                                ones_row[0:1, 0:128],
                                bv_sb[0:1, 0:FG],
                                start=False,
                                stop=True,
                            )
                            for hl in range(HPG):
                                nc.vector.tensor_copy(
                                    vp_g[:, rc * WV + 65 * hl:
                                         rc * WV + 65 * hl + 64],
                                    ps[:, hl * 64:(hl + 1) * 64],
                                )

            if do_proj:
                # wo load here overlaps attention; needed only at out proj
                wo_sb = wopool.tile([128, FCG * HID], MMDT, tag="wos")
                nc.sync.dma_start(
                    wo_sb[:].rearrange("p (f h) -> p f h", f=FCG),
                    woT[:, :].rearrange("(f p) h -> p f h", p=128),
                )

            # ---- attention: head pairs share the PE array via row groups ----
            with tc.tile_pool(name="sps", bufs=3, space="PSUM") as spool, \
                 tc.tile_pool(name="pvp", bufs=2, space="PSUM") as pvpool, \
                 tc.tile_pool(name="esb", bufs=2 if expmode == "f32r" else 3) as epool, \
                 tc.tile_pool(name="nrm", bufs=2) as npool, \
                 tc.tile_pool(name="pvs", bufs=2) as pvspool:
                for sqb in range(NSQB if do_attn else 0):
                    for hp in range(HPG // 2):
                        fcg = hp
                        q0 = qt_g[0:64,
                                  fcg * R + sqb * SQB: fcg * R + (sqb + 1) * SQB]
                        q1 = qt_g[64:128,
                                  fcg * R + sqb * SQB: fcg * R + (sqb + 1) * SQB]
                        pv0 = pvpool.tile([65, SQB], F32, tag="pv")
                        pv1 = pvpool.tile([65, SQB], F32, tag="pv")
                        pvs = [pv0, pv1]
                        for kch in range(NKC // 2):
                            sp0 = spool.tile([128, 2 * SQB], F32, tag="sp")
                            sp1 = spool.tile([128, 2 * SQB], F32, tag="sp")
                            sps = [sp0, sp1]
                            for j in range(2):
                                kc = 2 * kch + j
                                kslc = slice(fcg * S + kc * 128,
                                             fcg * S + kc * 128 + 128)
                                # heads 2hp (rows 0-63) and 2hp+1 (rows 64-127)
                                # run concurrently in disjoint PE row groups
                                nc.tensor.matmul(
                                    sps[0][:, j * SQB:(j + 1) * SQB],
                                    kt_g[0:64, kslc], q0, start=True, stop=True,
                                )
                                nc.tensor.matmul(
                                    sps[1][:, j * SQB:(j + 1) * SQB],
                                    kt_g[64:128, kslc], q1, start=True, stop=True,
                                )
                            ess = []
                            for h in range(2):
                                es = epool.tile([128, 2 * SQB], ESDT, tag="es")
                                nc.scalar.activation(es[:], sps[h][:], AF.Exp)
                                ess.append(es)
                            for j in range(2):
                                kc = 2 * kch + j
                                for h in range(2):
                                    hl = 2 * hp + h
                                    nc.tensor.matmul(
                                        pvs[h][:],
                                        vp_g[:, kc * WV + 65 * hl:
                                             kc * WV + 65 * hl + 65],
                                        ess[h][:, j * SQB:(j + 1) * SQB],
                                        start=(kc == 0),
                                        stop=(kc == NKC - 1),
                                    )
                        for h in range(2):
                            po = 64 * h
                            # copy PSUM->SBUF immediately to free the bank,
                            # then normalize off the SBUF copy
                            pv_sb = pvspool.tile([65, SQB], F32, tag="pvsb")
                            nc.vector.tensor_copy(pv_sb[:], pvs[h][:])
                            recip = npool.tile([1, SQB], F32, tag="recip")
                            nc.vector.reciprocal(recip[:], pv_sb[64:65, :])
                            rd = dbounce.tile([1, SQB], F32, tag="rd")
                            nc.sync.dma_start(rd[:], recip[:])
                            recipB = npool.tile([64, SQB], F32, tag="recipB")
                            nc.sync.dma_start(
                                recipB[:], rd[:].to_broadcast([64, SQB])
                            )
                            nc.vector.tensor_copy(
                                recipB[0:1, 0:1], recipB[0:1, 0:1]
                            )
                            nc.vector.tensor_mul(
                                h_tile[po:po + 64, fcg * R + sqb * SQB:
                                       fcg * R + (sqb + 1) * SQB],
                                pv_sb[0:64, :],
                                recipB[:],
                            )

            # ---- output projection: partial over this core's hidden dims ----
            OB = 512
            NOB = HID // OB       # 2
            with tc.tile_pool(name="ops", bufs=8, space="PSUM") as opool, \
                 tc.tile_pool(name="osb", bufs=4) as ospool:
                for quarter in range(R // 128 // 4 if do_out else 0):
                    pss = []
                    for _psi in range(4 * NOB):
                        ps_acc = opool.tile([128, OB], F32, tag="ps")
                        pss.append(ps_acc)
                    for fc in range(FCG):
                        for rl in range(4):
                            rc = quarter * 4 + rl
                            for ob in range(NOB):
                                nc.tensor.matmul(
                                    pss[rl * NOB + ob][:],
                                    h_tile[:, fc * R + rc * 128:
                                           fc * R + rc * 128 + 128],
                                    wo_sb[:, fc * HID + ob * OB:
                                          fc * HID + (ob + 1) * OB],
                                    start=(fc == 0),
                                    stop=(fc == FCG - 1),
                                )
                    for rl in range(4):
                        rc = quarter * 4 + rl
                        for ob in range(NOB):
                            o_sb = ospool.tile([128, OB], F32, tag="o")
                            nc.vector.tensor_copy(o_sb[:], pss[rl * NOB + ob][:])
                            nc.sync.dma_start(
                                out[rc * 128:(rc + 1) * 128,
                                    ob * OB:(ob + 1) * OB],
                                o_sb[:],
                            )
    return nc



def build4(nc: bass.Bass, cfg: Cfg3, reps: int = 1, expmode: str = "f32r"):
    """Interleaved schedule: KV proj first, then per-query-block loop where
    Q projection (next block) and output projection (previous block) are
    emitted between attention head-pair blocks so PE fills the gaps while
    the Activation engine streams exp. expmode "f32r": exp writes f32 and
    PV matmuls run f32r (avoids the slow PSUM->bf16 activation path);
    "bf16": exp writes bf16, PV in bf16."""
    HID, R, S = cfg.HID, cfg.R, cfg.S
    IC, HPG, FG, FCG = cfg.IC, cfg.HPG, cfg.FG, cfg.FCG
    NKC, SQB, NSQB, WV, XW = cfg.NKC, cfg.SQB, cfg.NSQB, cfg.WV, cfg.XW
    MMDT = cfg.MMDT
    F32R = DT.float32r
    ESDT = F32R if expmode == "f32r" else MMDT
    VPDT = F32R if expmode == "f32r" else MMDT

    dp = nc.declare_dram_parameter
    qT = dp("qT", [HID, R], MMDT, isOutput=False)
    kT = dp("kT", [HID, S], MMDT, isOutput=False)
    vT = dp("vT", [HID, S], MMDT, isOutput=False)
    wqT = dp("wqT", [HID, FG], MMDT, isOutput=False)
    wkT = dp("wkT", [HID, FG], MMDT, isOutput=False)
    wvT = dp("wvT", [HID, FG], MMDT, isOutput=False)
    woT = dp("woT", [FG, HID], MMDT, isOutput=False)
    bqr = dp("bqr", [1, FG], MMDT, isOutput=False)
    bkr = dp("bkr", [1, FG], MMDT, isOutput=False)
    bvr = dp("bvr", [1, FG], MMDT, isOutput=False)
    maskf = dp("maskf", [1, R], F32, isOutput=False)
    out = dp("out", [R, HID], F32, isOutput=True)

    def pv_ops(a, b):
        return a, b

    with tile.TileContext(nc) as tc, ExitStack() as ctx:
        cpool = ctx.enter_context(tc.tile_pool(name="consts", bufs=1))
        bq_sb = cpool.tile([1, FG], MMDT, tag="bq")
        bk_sb = cpool.tile([1, FG], MMDT, tag="bk")
        bv_sb = cpool.tile([1, FG], MMDT, tag="bv")
        nc.sync.dma_start(bq_sb[:], bqr[:])
        nc.sync.dma_start(bk_sb[:], bkr[:])
        nc.sync.dma_start(bv_sb[:], bvr[:])
        ones_f32 = cpool.tile([1, XW], F32, tag="ones32")
        nc.vector.memset(ones_f32[:], 1.0)
        ones_row = cpool.tile([1, XW], MMDT, tag="ones")
        nc.vector.tensor_copy(ones_row[:], ones_f32[:])
        NOC = NKC * HPG
        onesw_f32 = cpool.tile([128, NOC], F32, tag="onesw32")
        nc.vector.memset(onesw_f32[:], 1.0)
        ones_wide = cpool.tile([128, NOC], VPDT, tag="onesw")
        nc.vector.tensor_copy(ones_wide[:], onesw_f32[:])
        mrow_f32 = cpool.tile([1, R], F32, tag="mrow32")
        nc.sync.dma_start(mrow_f32[:], maskf[:])
        mrow = cpool.tile([1, R], MMDT, tag="mrow")
        nc.vector.tensor_copy(mrow[:], mrow_f32[:])
        dbounce = ctx.enter_context(
            tc.tile_pool(name="dbounce", bufs=4, space="DRAM"))

        gpool = ctx.enter_context(tc.tile_pool(name="gstore", bufs=1))
        h_tile = gpool.tile([128, FCG * R], MMDT, tag="h")

        wpool = ctx.enter_context(tc.tile_pool(name="wgt", bufs=2))
        xpool = ctx.enter_context(tc.tile_pool(name="xin", bufs=2))
        wqpool = ctx.enter_context(tc.tile_pool(name="wq", bufs=1))
        xqpool = ctx.enter_context(tc.tile_pool(name="xq", bufs=1))
        wopool = ctx.enter_context(tc.tile_pool(name="wo", bufs=1))

        def pe_touch(ppool, ap):
            pt = ppool.tile([1, 1], F32, tag="pt", bufs=2)
            nc.tensor.matmul(pt[:], ap, ap, start=True, stop=True)

        def load_w(wT, ppool):
            w_sb = wpool.tile([128, IC * FG], MMDT, tag="w")
            src = wT[:, :].rearrange("(i p) f -> p i f", p=128)
            nc.sync.dma_start(w_sb[:].rearrange("p (i f) -> p i f", i=IC), src)
            pe_touch(ppool, w_sb[0:1, 0:1])
            return w_sb

        def load_x(xT, rb, ppool):
            x_sb = xpool.tile([128, IC * XW], MMDT, tag="x")
            src = xT[:, rb * XW:(rb + 1) * XW].rearrange(
                "(i p) w -> p i w", p=128)
            nc.sync.dma_start(x_sb[:].rearrange("p (i w) -> p i w", i=IC), src)
            pe_touch(ppool, x_sb[0:1, 0:1])
            return x_sb

        for _rep in range(reps):
            qt_g = gpool.tile([128, FCG * R], MMDT, tag="qt")
            kt_g = gpool.tile([128, FCG * S], MMDT, tag="kt")
            vp_g = gpool.tile([128, NKC * WV], VPDT, tag="vp")
            nc.vector.tensor_copy(
                vp_g[:].rearrange("p (a e) -> p a e", e=65)[:, :, 64:65],
                ones_wide[:].unsqueeze(2),
            )

            with tc.tile_pool(name="pp", bufs=3, space="PSUM") as ppool:
                # ---- K projection ----
                w_sb = load_w(wkT, ppool)
                for rb in range(S // XW):
                    x_sb = load_x(kT, rb, ppool)
                    for fcg in range(FCG):
                        ps = ppool.tile([128, XW], F32, tag="ps")
                        for ic in range(IC):
                            nc.tensor.matmul(
                                ps[:],
                                w_sb[:, ic * FG + fcg * 128:
                                     ic * FG + fcg * 128 + 128],
                                x_sb[:, ic * XW:(ic + 1) * XW],
                                start=(ic == 0),
                                stop=False,
                            )
                        nc.tensor.matmul(
                            ps[:],
                            bk_sb[0:1, fcg * 128:(fcg + 1) * 128],
                            ones_row[0:1, 0:XW],
                            start=False,
                            stop=True,
                        )
                        nc.vector.tensor_copy(
                            kt_g[:, fcg * S + rb * XW:
                                 fcg * S + (rb + 1) * XW],
                            ps[:],
                        )

                # ---- V projection + ones column ----
                w_sb = load_w(wvT, ppool)
                NRC = XW // 128
                for rb4 in range(S // XW):
                    x_sb = load_x(vT, rb4, ppool)
                    for rcl in range(NRC):
                        rc = rb4 * NRC + rcl
                        ps = ppool.tile([128, FG], F32, tag="ps")
                        for ic in range(IC):
                            nc.tensor.matmul(
                                ps[:],
                                x_sb[:, ic * XW + rcl * 128:
                                     ic * XW + rcl * 128 + 128],
                                w_sb[:, ic * FG:(ic + 1) * FG],
                                start=(ic == 0),
                                stop=False,
                            )
                        nc.tensor.matmul(
                            ps[:],
                            ones_row[0:1, 0:128],
                            bv_sb[0:1, 0:FG],
                            start=False,
                            stop=True,
                        )
                        for hl in range(HPG):
                            nc.vector.tensor_copy(
                                vp_g[:, rc * WV + 65 * hl:
                                     rc * WV + 65 * hl + 64],
                                ps[:, hl * 64:(hl + 1) * 64],
                            )

                # ---- resident Q weights + all Q input blocks + wo ----
                wq_sb = wqpool.tile([128, IC * FG], MMDT, tag="wqs")
                nc.sync.dma_start(
                    wq_sb[:].rearrange("p (i f) -> p i f", i=IC),
                    wqT[:, :].rearrange("(i p) f -> p i f", p=128),
                )
                pe_touch(ppool, wq_sb[0:1, 0:1])
                xq_all = xqpool.tile([128, IC * R], MMDT, tag="xqa")
                nc.sync.dma_start(
                    xq_all[:].rearrange("p (i w) -> p i w", i=IC),
                    qT[:, :].rearrange("(i p) w -> p i w", p=128),
                )
                pe_touch(ppool, xq_all[0:1, 0:1])
                wo_sb = wopool.tile([128, FCG * HID], MMDT, tag="wos")
                nc.sync.dma_start(
                    wo_sb[:].rearrange("p (f h) -> p f h", f=FCG),
                    woT[:, :].rearrange("(f p) h -> p f h", p=128),
                )

                # ---- Q projection for sqb 0 (rest interleaved below) ----
                def qproj_block(pool, sqb, fcgs):
                    for fcg in fcgs:
                        ps = pool.tile([128, SQB], F32, tag="qo")
                        for ic in range(IC):
                            nc.tensor.matmul(
                                ps[:],
                                wq_sb[:, ic * FG + fcg * 128:
                                      ic * FG + fcg * 128 + 128],
                                xq_all[:, ic * R + sqb * SQB:
                                       ic * R + (sqb + 1) * SQB],
                                start=(ic == 0),
                                stop=False,
                            )
                        nc.tensor.matmul(
                            ps[:],
                            bq_sb[0:1, fcg * 128:(fcg + 1) * 128],
                            mrow[0:1, sqb * SQB:(sqb + 1) * SQB],
                            start=False,
                            stop=True,
                        )
                        nc.vector.tensor_copy(
                            qt_g[:, fcg * R + sqb * SQB:
                                 fcg * R + (sqb + 1) * SQB],
                            ps[:],
                        )

                qproj_block(ppool, 0, range(FCG))

            # ---- attention + interleaved Q proj / out proj ----
            OB = 512
            NOB = HID // OB

            with tc.tile_pool(name="sps", bufs=2, space="PSUM") as spool, \
                 tc.tile_pool(name="pvp", bufs=2, space="PSUM") as pvpool, \
                 tc.tile_pool(name="sml", bufs=2, space="PSUM") as smpool, \
                 tc.tile_pool(name="esb", bufs=2) as epool, \
                 tc.tile_pool(name="nrm", bufs=2) as npool, \
                 tc.tile_pool(name="osb", bufs=2) as ospool, \
                 tc.tile_pool(name="pvs", bufs=2) as pvspool:

                def out_block(sqb, ob):
                    for rl in range(SQB // 128):
                        rc = sqb * (SQB // 128) + rl
                        ps = smpool.tile([128, OB], F32, tag="qo")
                        for fc in range(FCG):
                            nc.tensor.matmul(
                                ps[:],
                                h_tile[:, fc * R + rc * 128:
                                       fc * R + rc * 128 + 128],
                                wo_sb[:, fc * HID + ob * OB:
                                      fc * HID + (ob + 1) * OB],
                                start=(fc == 0),
                                stop=(fc == FCG - 1),
                            )
                        o_sb = ospool.tile([128, OB], F32, tag="o")
                        nc.vector.tensor_copy(o_sb[:], ps[:])
                        nc.sync.dma_start(
                            out[rc * 128:(rc + 1) * 128,
                                ob * OB:(ob + 1) * OB],
                            o_sb[:],
                        )

                for sqb in range(NSQB):
                    for hp in range(HPG // 2):
                        fcg = hp
                        q0 = qt_g[0:64,
                                  fcg * R + sqb * SQB: fcg * R + (sqb + 1) * SQB]
                        q1 = qt_g[64:128,
                                  fcg * R + sqb * SQB: fcg * R + (sqb + 1) * SQB]
                        pv0 = pvpool.tile([65, SQB], F32, tag="pv")
                        pv1 = pvpool.tile([65, SQB], F32, tag="pv")
                        pvs = [pv0, pv1]
                        for kch in range(NKC // 2):
                            sp0 = spool.tile([128, 2 * SQB], F32, tag="sp")
                            sp1 = spool.tile([128, 2 * SQB], F32, tag="sp")
                            sps = [sp0, sp1]
                            for j in range(2):
                                kc = 2 * kch + j
                                kslc = slice(fcg * S + kc * 128,
                                             fcg * S + kc * 128 + 128)
                                nc.tensor.matmul(
                                    sps[0][:, j * SQB:(j + 1) * SQB],
                                    kt_g[0:64, kslc], q0, start=True, stop=True,
                                )
                                nc.tensor.matmul(
                                    sps[1][:, j * SQB:(j + 1) * SQB],
                                    kt_g[64:128, kslc], q1, start=True, stop=True,
                                )
                            ess = []
                            for h in range(2):
                                es = epool.tile([128, 2 * SQB], ESDT, tag="es")
                                nc.scalar.activation(es[:], sps[h][:], AF.Exp)
                                ess.append(es)
                            for j in range(2):
                                kc = 2 * kch + j
                                for h in range(2):
                                    hl = 2 * hp + h
                                    a_op, b_op = pv_ops(
                                        vp_g[:, kc * WV + 65 * hl:
                                             kc * WV + 65 * hl + 65],
                                        ess[h][:, j * SQB:(j + 1) * SQB],
                                    )
                                    nc.tensor.matmul(
                                        pvs[h][:],
                                        a_op,
                                        b_op,
                                        start=(kc == 0),
                                        stop=(kc == NKC - 1),
                                    )
                        for h in range(2):
                            po = 64 * h
                            pv_sb = pvspool.tile([65, SQB], F32, tag="pvsb")
                            nc.vector.tensor_copy(pv_sb[:], pvs[h][:])
                            recip = npool.tile([1, SQB], F32, tag="recip")
                            nc.vector.reciprocal(recip[:], pv_sb[64:65, :])
                            rd = dbounce.tile([1, SQB], F32, tag="rd")
                            nc.sync.dma_start(rd[:], recip[:])
                            recipB = npool.tile([64, SQB], F32, tag="recipB")
                            nc.sync.dma_start(
                                recipB[:], rd[:].to_broadcast([64, SQB])
                            )
                            nc.vector.tensor_copy(
                                recipB[0:1, 0:1], recipB[0:1, 0:1]
                            )
                            nc.vector.tensor_mul(
                                h_tile[po:po + 64, fcg * R + sqb * SQB:
                                       fcg * R + (sqb + 1) * SQB],
                                pv_sb[0:64, :],
                                recipB[:],
                            )
                        # interleaved PE filler: next Q block, prev out rows
                        if hp == 0 and sqb + 1 < NSQB:
                            qproj_block(smpool, sqb + 1, (0, 1))
                        elif hp == 1 and sqb + 1 < NSQB:
                            qproj_block(smpool, sqb + 1, (2, 3))
                        elif hp == 2 and sqb >= 1:
                            out_block(sqb - 1, 0)
                        elif hp == 3 and sqb >= 1:
                            out_block(sqb - 1, 1)

                out_block(NSQB - 1, 0)
                out_block(NSQB - 1, 1)
    return nc


_compiled = {}


SCHED = "v4"      # "v3" | "v4"
EXPMODE = "f32r"  # "f32r" | "bf16"


def _get_nc(cfg_key):
    if cfg_key not in _compiled:
        cfg = Cfg3(HID=HID, NH=NH, S=S, use_bf16=USE_BF16, use_f32r=USE_F32R)
        nc = bacc.Bacc(
            "TRN2", target_bir_lowering=False, debug=False, num_devices=N_CORES
        )
        if SCHED == "v4":
            build4(nc, cfg, expmode=EXPMODE)
        else:
            build3(nc, cfg, expmode=EXPMODE)
        nc.compile()
        _compiled[cfg_key] = (nc, cfg)
    return _compiled[cfg_key]


def make_in_maps(cfg, q, k, v, mask, wq, bq, wk, bk, wv, bv, wo, bo):
    """Per-core input maps for head-parallel sharding."""
    f32 = np.float32
    mmnp = mybir.dt.np(cfg.MMDT) if USE_BF16 else f32
    scale = f32(1.0 / np.sqrt(DH))
    FG = cfg.FG
    mrow_b = [((np.asarray(mask[b]) != 0).astype(f32) * scale)
              for b in range(B)]
    qT_b = [(np.asarray(q[b], f32) * mrow_b[b][:, None]).T.astype(mmnp)
            for b in range(B)]
    kT_b = [np.asarray(k[b], f32).T.astype(mmnp) for b in range(B)]
    vT_b = [np.asarray(v[b], f32).T.astype(mmnp) for b in range(B)]
    mask_b = [mrow_b[b].reshape(1, S) for b in range(B)]
    wqT = np.asarray(wq, f32).T
    wkT = np.asarray(wk, f32).T
    wvT = np.asarray(wv, f32).T
    woT = np.asarray(wo, f32).T
    in_maps = []
    for c in range(N_CORES):
        b, hg = c // 2, c % 2
        hs = slice(hg * FG, (hg + 1) * FG)
        m = {
            "qT": qT_b[b],
            "kT": kT_b[b],
            "vT": vT_b[b],
            "maskf": mask_b[b],
            "wqT": wqT[:, hs].astype(mmnp),
            "wkT": wkT[:, hs].astype(mmnp),
            "wvT": wvT[:, hs].astype(mmnp),
            "woT": woT[hs, :].astype(mmnp),
            "bqr": (np.asarray(bq, f32)[hs] * scale
                    ).reshape(1, FG).astype(mmnp),
            "bkr": np.asarray(bk, f32)[hs].reshape(1, FG).astype(mmnp),
            "bvr": np.asarray(bv, f32)[hs].reshape(1, FG).astype(mmnp),
        }
        in_maps.append(m)
    return in_maps


def kernel(q, k, v, mask, wq, bq, wk, bk, wv, bv, wo, bo):
    q = np.asarray(q, dtype=np.float32)
    k = np.asarray(k, dtype=np.float32)
    v = np.asarray(v, dtype=np.float32)
    mask = np.asarray(mask)
    f32 = np.float32

    nc, cfg = _get_nc((USE_F32R, USE_BF16, SHARD, SCHED, EXPMODE))
    in_maps = make_in_maps(cfg, q, k, v, mask, wq, bq, wk, bk, wv, bv, wo, bo)

    res = run_bass_kernel_spmd(nc, in_maps, list(range(N_CORES)), trace=TRACE)
    LAST_RESULTS[0] = res

    out = np.empty((B, S, HID), dtype=np.float32)
    bof = np.asarray(bo, f32).reshape(1, HID)
    for b in range(B):
        out[b] = res.results[2 * b]["out"]
        out[b] += res.results[2 * b + 1]["out"]
        out[b] += bof
    return out
